# revision 25
# baseline (speedup 1.0000x reference)
"""Discrete mixture (MoE-style routing) Bass kernel for Trainium2.

Reference computation (per batch row b):
    logits  = params[b, :K]
    gumbel  = -log(-log(uniform_noise[b]))
    sel     = argmax(logits + gumbel)                      # categorical sample
    comp    = params[b, K + sel*2D : K + (sel+1)*2D]       # gather routed expert params
    mean, log_std = comp[:D], comp[D:]
    out[b]  = mean + exp(log_std) * eps[b]

Sharding: pure data parallel over the batch axis across 8 NeuronCores
(128 rows per core, one row per SBUF partition).

Precision: component params and eps are shipped bf16 (round-to-nearest host
cast - pure data marshaling), and the output is stored bf16 and upcast to
fp32 on the host. Halves gather + eps + store HBM traffic and runs the DVE
mult/add tail at 2x rate. Routing (logits + gumbel + argmax) stays fp32 so
the selected component matches the fp32 reference exactly (bf16 logits
would flip near-tie argmaxes -> O(1) row errors). Measured rel err ~9e-3
(gate 2e-2).

Raw bacc (no TileContext), hand-placed semaphores. Measured phase map
(~23us total; NEFF preamble ~6.3-6.9us and postamble ~1.45us are fixed
framework/runtime cost, as are the const-AP MEMSETs - they are emitted
unconditionally in Bass.__init__):
  body+0.0  aux DMA issue (sync); ACT table load (set 6 = natural_log_exp
            covers Ln AND Exp in one ~1.3us load)
  body+2.3  aux receipt -> ACT ln(-ln u) x2 -> DVE sub/max8/find_index8/
            scalar_tensor_tensor offs = idx*2D + (row*TOTAL + base)
  body+4.1  Pool/SWDGE: two 1-offset-per-row contiguous indirect gathers
            (128 x 4KB descriptors each; ~1.1us gen each, serialized on
            Q7): ls first - it feeds exp -> mult -> add; mean's data
            drains behind exp/mult compute
  body+8.3  ls lands -> ACT exp x2 (1024 cols each) -> DVE mult0, mult1,
            add0, add1 (all bf16 2x rate) -> ACT store x2 (qScalar HWDGE)
  ~ +13.2   last store receipt -> sem_clear -> postamble

Key scheduling facts (HW-measured):
- SWDGE gen is ~1us fixed + 0.34ns/descriptor, so exactly two gathers
  (a third split costs another serialized ~1.1us gen); contiguous 4KB
  descriptors drain at SDMA line rate (158ns/4KB/engine).
- The mean gather's completion consistently lands after exp1 finishes,
  so the DVE order mult0, mult1, add0, add1 never stalls; add0-before-
  mult1 loses ~0.7us whenever mean is late (SDMA engines 7/15 straggle
  on the second gather - known SWDGE descriptor-ring contention).
- Queue warm-ups: a 4B-per-partition gpsimd load arms the SWDGE path +
  queue-0 rings while Q7 idles; an ACT store of the zero column back to
  the consumed aux DRAM tensor (data no-op, race-free) arms the qScalar
  store ring during ACT's idle window (first-use issue is ~1.3us vs
  ~0.6us warm). Neither may gate the Ln chain: making ACT wait on the
  gpsimd warm cost 2.9us once.
- Offsets stay exact through the fp32 DVE path: every offset is a
  multiple of 4 and < 2^26, where fp32 spacing is <= 4.

Avoid (all HW-verified on this problem):
- tensor_tensor_reduce (either max or min reduce): hangs the device
  (NRT_EXEC_UNIT_UNRECOVERABLE) - keep sub/max8/find_index8 separate.
- A third gather split; GPSIMD for elementwise (SBUF-port contention
  with DVE slows both ~2x); stores on the SWDGE path (slow drain tail);
  fp32 adds/stores (slower DVE + 2x store bytes for ~3e-3 less rel err);
  warm stores targeting the out tensor (intermittent NaN when the 4B
  garbage write raced the real store).
"""

import numpy as np
import ml_dtypes

import concourse.bacc as bacc
import concourse.bass as bass
from concourse import mybir
from concourse.bass_utils import run_bass_kernel_spmd

AF = mybir.ActivationFunctionType
ALU = mybir.AluOpType
BF16 = mybir.dt.bfloat16
F32 = mybir.dt.float32

B = 1024
K = 64
D = 2048
TWO_D = 2 * D
TOTAL = K + K * TWO_D  # 262208
N_CORES = 8
ROWS = B // N_CORES  # 128 rows per core == SBUF partition count

EXP_SPLITS = [(0, 1024), (1024, 2048)]  # exp/mult/add/store chunks
# gather base offsets packed into aux, in issue order: ls, mean
GATHER_BASES = [K + D, K]
N_G = len(GATHER_BASES)
AUX_W = 2 * K + N_G + 1  # + zeros column for activation bias

# natural_log_exp_and_others: one ACT table set covering both Ln and Exp
ACT_SET_LN_EXP = 6

_CACHE: dict = {}


def _build_program() -> bass.Bass:
    nc = bacc.Bacc("TRN2", target_bir_lowering=False, debug=False)

    params = nc.dram_tensor("params", [ROWS, TOTAL], BF16, kind="ExternalInput").ap()
    aux = nc.dram_tensor(
        "aux", [ROWS, AUX_W], mybir.dt.uint32, kind="ExternalInput"
    ).ap()
    eps = nc.dram_tensor("eps", [ROWS, D], BF16, kind="ExternalInput").ap()
    out = nc.dram_tensor("out", [ROWS, D], BF16, kind="ExternalOutput").ap()

    # SBUF tiles
    aux_t = nc.alloc_sbuf_tensor("aux_t", [ROWS, AUX_W], mybir.dt.uint32).ap()
    eps_t = nc.alloc_sbuf_tensor("eps_t", [ROWS, D], BF16).ap()
    t1 = nc.alloc_sbuf_tensor("t1", [ROWS, K], F32).ap()
    scores = nc.alloc_sbuf_tensor("scores", [ROWS, K], F32).ap()
    max8 = nc.alloc_sbuf_tensor("max8", [ROWS, 8], F32).ap()
    idx8 = nc.alloc_sbuf_tensor("idx8", [ROWS, 8], mybir.dt.uint32).ap()
    offs = nc.alloc_sbuf_tensor("offs", [ROWS, N_G], mybir.dt.uint32).ap()
    ls_t = nc.alloc_sbuf_tensor("ls_t", [ROWS, D], BF16).ap()
    mean_t = nc.alloc_sbuf_tensor("mean_t", [ROWS, D], BF16).ap()
    std = nc.alloc_sbuf_tensor("std", [ROWS, D], BF16).ap()
    res = nc.alloc_sbuf_tensor("res", [ROWS, D], BF16).ap()
    res_o = nc.alloc_sbuf_tensor("res_o", [ROWS, D], BF16).ap()
    warm_t = nc.alloc_sbuf_tensor("warm_t", [ROWS, 2], BF16).ap()

    # semaphores (allocated contiguously; cleared as one range at the end)
    s_aux = nc.alloc_semaphore("s_aux")
    s_eps = nc.alloc_semaphore("s_eps")
    s_lnln = nc.alloc_semaphore("s_lnln")
    s_offs = nc.alloc_semaphore("s_offs")
    s_ls = nc.alloc_semaphore("s_ls")
    s_m = nc.alloc_semaphore("s_m")
    s_exp = nc.alloc_semaphore("s_exp")
    s_add = nc.alloc_semaphore("s_add")
    s_st = nc.alloc_semaphore("s_st")
    s_warm = nc.alloc_semaphore("s_warm")
    # per-engine tick sems guarding same-engine RAW hazards: engines pipeline
    # consecutive instructions, so op N+1 can read SBUF before op N's
    # writeback lands unless it waits on a sem N increments at completion
    s_dve = nc.alloc_semaphore("s_dve")
    s_act = nc.alloc_semaphore("s_act")
    sem_nums = [
        s.num
        for s in (s_aux, s_eps, s_lnln, s_offs, s_ls, s_m, s_exp, s_add,
                  s_st, s_warm, s_dve, s_act)
    ]
    assert max(sem_nums) - min(sem_nums) + 1 == len(sem_nums), sem_nums

    noise_v = aux_t[:, 0:K].bitcast(F32)
    logits_v = aux_t[:, K : 2 * K].bitcast(F32)
    bases_v = aux_t[:, 2 * K : 2 * K + N_G]
    zero_bias = aux_t[:, 2 * K + N_G : 2 * K + N_G + 1].bitcast(F32)

    # ---- SP (sync): input DMAs, then idle (its teardown overlaps) ----
    nc.sync.dma_start(out=aux_t[:], in_=aux[:]).then_inc(s_aux, 16)
    nc.sync.dma_start(out=eps_t[:], in_=eps[:]).then_inc(s_eps, 16)

    # ---- Pool: warm the SWDGE path + queue-0 SDMA rings with a tiny
    # 4B-per-partition load while Q7 is otherwise idle, so the real
    # gathers skip the first-use arming latency.
    nc.gpsimd.dma_start(out=warm_t[:], in_=params[:, 0:2]).then_inc(s_warm, 16)

    # ---- ACT: table load, gumbel lns (zero_bias avoids const-AP memsets) --
    nc.scalar.add_instruction(
        mybir.InstLoadActFuncSet(
            name=nc.get_next_instruction_name(),
            engine=mybir.EngineType.Activation,
            act_func_set_id=ACT_SET_LN_EXP,
            ins=[],
            outs=[],
        )
    )
    nc.scalar.wait_ge(s_aux, 16)
    nc.scalar.activation(t1[:], noise_v, AF.Ln, bias=zero_bias).then_inc(s_act, 1)
    nc.scalar.wait_ge(s_act, 1)  # t1 RAW
    nc.scalar.activation(
        t1[:], t1[:], AF.Ln, bias=zero_bias, scale=-1.0
    ).then_inc(s_lnln, 1)
    # warm the ACT HWDGE store queue during its idle window (first-use
    # sequencer arming is ~1.3us vs ~0.6us warm): write the zero column
    # back to the already-consumed aux DRAM tensor - a data no-op
    nc.scalar.dma_start(
        out=aux[0:1, AUX_W - 1 : AUX_W], in_=aux_t[0:1, AUX_W - 1 : AUX_W]
    ).then_inc(s_warm, 16)

    # ---- DVE: routing (tick-sem guarded chain) ----
    # Offsets stay exact through the fp32 ALU path: every offset is a
    # multiple of 4 and < 2^26, where fp32 spacing is <= 4.
    nc.vector.wait_ge(s_lnln, 1)  # implies s_aux >= 16 (ACT waited first)
    nc.vector.tensor_tensor(
        out=scores[:], in0=logits_v, in1=t1[:], op=ALU.subtract
    ).then_inc(s_dve, 1)
    nc.vector.wait_ge(s_dve, 1)
    nc.vector.max(max8[:], scores[:]).then_inc(s_dve, 1)
    nc.vector.wait_ge(s_dve, 2)
    nc.vector.max_index(idx8[:], max8[:], scores[:]).then_inc(s_dve, 1)
    nc.vector.wait_ge(s_dve, 3)
    nc.vector.scalar_tensor_tensor(
        out=offs[:],
        in0=idx8[:, 0:1].to_broadcast([ROWS, N_G]),
        scalar=TWO_D,
        in1=bases_v,
        op0=ALU.mult,
        op1=ALU.add,
    ).then_inc(s_offs, 1)

    # ---- Pool: two 1-offset-per-row contiguous indirect gathers (128
    # descriptors of 4KB each, one SWDGE gen each). ls first - it feeds the
    # long exp -> mult -> add chain; mean's data+receipt hides behind the
    # exp/mult compute.
    def gather(dst, off_col, sem):
        nc.gpsimd.indirect_dma_start(
            out=dst,
            out_offset=None,
            in_=params[:, :],
            in_offset=bass.IndirectOffsetOnAxis(ap=off_col, axis=1),
        ).then_inc(sem, 16)

    nc.gpsimd.wait_ge(s_offs, 1)
    gather(ls_t[:, :], offs[:, 0:1], s_ls)
    gather(mean_t[:, :], offs[:, 1:2], s_m)

    # ---- ACT: exp chunks (bf16 out) ----
    nc.scalar.wait_ge(s_ls, 16)
    for s, e in EXP_SPLITS:
        nc.scalar.activation(
            std[:, s:e], ls_t[:, s:e], AF.Exp, bias=zero_bias
        ).then_inc(s_exp, 1)

    # ---- DVE: bf16 mults and adds (2x rate), ordered mult0, mult1, add0,
    # add1: the mean gather's receipt consistently lands after exp1, so
    # running both mults first keeps DVE busy until mean arrives and the
    # adds then run back-to-back. s_dve ticks: mult0 -> 4, mult1 -> 5
    # (adds inc only s_add; adds have no RAW among themselves).
    def mult(i):
        s, e = EXP_SPLITS[i]
        nc.vector.wait_ge(s_exp, i + 1)
        if i == 0:
            nc.vector.wait_ge(s_eps, 16)
        nc.vector.tensor_tensor(
            out=res[:, s:e], in0=std[:, s:e], in1=eps_t[:, s:e], op=ALU.mult
        ).then_inc(s_dve, 1)

    def add(i, dve_tick):
        s, e = EXP_SPLITS[i]
        nc.vector.wait_ge(s_dve, dve_tick)
        nc.vector.tensor_tensor(
            out=res_o[:, s:e], in0=res[:, s:e], in1=mean_t[:, s:e], op=ALU.add
        ).then_inc(s_add, 1)

    mult(0)
    mult(1)  # -> tick 5
    nc.vector.wait_ge(s_m, 16)
    add(0, 4)  # (0,1024): mult0 + mean        -> s_add 1
    add(1, 5)  # (1024,2048): mult1 + mean     -> s_add 2

    # ---- ACT: stores (qActDynamicHW ring), final wait, sem cleanup ----
    for i in range(len(EXP_SPLITS)):
        s, e = EXP_SPLITS[i]
        nc.scalar.wait_ge(s_add, i + 1)
        nc.scalar.dma_start(out=out[:, s:e], in_=res_o[:, s:e]).then_inc(s_st, 16)
    nc.scalar.wait_ge(s_st, 16 * len(EXP_SPLITS))
    nc.scalar.sem_clear(range(min(sem_nums), max(sem_nums) + 1))

    nc.finalize()
    return nc


def _get_program() -> bass.Bass:
    if "nc" not in _CACHE:
        _CACHE["nc"] = _build_program()
    return _CACHE["nc"]


def make_in_maps(params, uniform_noise, eps):
    params = np.ascontiguousarray(params, dtype=np.float32)
    uniform_noise = np.ascontiguousarray(uniform_noise, dtype=np.float32)
    eps_bf = np.ascontiguousarray(eps, dtype=np.float32).astype(ml_dtypes.bfloat16)
    params_bf = params.astype(ml_dtypes.bfloat16)
    row = np.arange(ROWS, dtype=np.uint64) * TOTAL
    in_maps = []
    for i in range(N_CORES):
        sl = slice(i * ROWS, (i + 1) * ROWS)
        aux = np.empty((ROWS, AUX_W), np.uint32)
        aux[:, 0:K] = uniform_noise[sl].view(np.uint32)
        aux[:, K : 2 * K] = np.ascontiguousarray(params[sl, :K]).view(np.uint32)
        for g, base in enumerate(GATHER_BASES):
            aux[:, 2 * K + g] = (row + base).astype(np.uint32)
        aux[:, 2 * K + N_G] = 0  # fp32 0.0 bits: activation bias column
        in_maps.append(
            {
                "params": params_bf[sl],
                "aux": aux,
                "eps": eps_bf[sl],
            }
        )
    return in_maps


def kernel(params, uniform_noise, eps, **run_kwargs):
    nc = _get_program()
    in_maps = make_in_maps(params, uniform_noise, eps)
    res = run_bass_kernel_spmd(nc, in_maps, list(range(N_CORES)), **run_kwargs)
    out = np.concatenate(
        [np.asarray(r["out"]).astype(np.float32) for r in res.results], axis=0
    )
    if run_kwargs:
        _CACHE["last_results"] = res
    return out
